# revision 29
# baseline (speedup 1.0000x reference)
"""Trainium2 Bass kernel for CanonCausalMultiheadAttn (v3).

Sharding: tensor-parallel over heads across 8 cores (2 q-heads + 1 kv-head
per core), both batches replicated. Attention outputs are exchanged with
TWO AllToAlls (one per local q-head) so round 0 overlaps round-1 compute;
each core then owns one (batch, 512-seq-slice) of the output projection.

Per-core pipeline (B=2, S=2048, D=2048 hardcoded):
  QKV proj: bf16 matmuls, the PE queue is pure matmuls plus tiny norm
  reductions -> canon conv on full bf16 rows (DVE taps at 2x 16-bit rate,
  first tap folded into the ACT psum-drain copy) -> qk rmsnorm:
  sum-of-squares via ones-column matmuls, q-rstd broadcast via ones-row
  matmul + wide 128-lane reciprocal (no single-partition reciprocals
  anywhere), k-rstd transposed via DRAM and applied as the EXP
  per-partition scale -> RoPE (bf16 DVE, norm weight & 1/sqrt(dh) folded
  into bf16 tables) -> causal attention with scores in [Sk, Sq] layout;
  per K-block one wide matmul row + one wide EXP -> PV swapped
  (stationary = V block, moving = P) emitting O'^T [dh, q] directly in
  a2a layout, denominators via ones-column rowsum matmuls, normalization
  via broadcast matmul + wide reciprocal folded into the psum drain ->
  AllToAll x2 -> output projection (stationary = received attn block,
  resident Wo in SBUF). DMA triggers are spread across engine queues
  (weights/activations on Sync, V transposes on GpSimd, rope shift on
  Vector, norm roundtrip + tables on Scalar) to avoid head-of-line
  blocking of the weight stream.
"""
import sys

sys.path.insert(0, '/opt/trn_rl_repo')

import numpy as np
import ml_dtypes

import concourse.bass as bass
import concourse.mybir as mybir
import concourse.tile as tile
from concourse import bacc
from concourse.bass_utils import run_bass_kernel_spmd

F32 = mybir.dt.float32
F32R = mybir.dt.float32r
BF16 = mybir.dt.bfloat16
AF = mybir.ActivationFunctionType
ALU = mybir.AluOpType

B, S, D = 2, 2048, 2048
NH, NKV, DH = 16, 8, 128
K_CONV = 4
EPS = 1e-6
SCALE = 1.0 / float(np.sqrt(DH))
NEG = -1e9
N_CORES = 8
NCH = S // 512          # 512-wide seq chunks
NI = S // 128           # 128-wide Sk blocks


def _build():
    nc = bacc.Bacc("TRN2", target_bir_lowering=False, debug=False,
                   num_devices=N_CORES)

    hsT = nc.dram_tensor("hsT", [D, B * S], BF16, kind="ExternalInput")
    wT = nc.dram_tensor("wT", [D, 512], BF16, kind="ExternalInput")
    woT = nc.dram_tensor("woT", [D, D], BF16, kind="ExternalInput")
    cw = nc.dram_tensor("cw", [512, K_CONV], F32, kind="ExternalInput")
    ropeAq = nc.dram_tensor("ropeAq", [DH, S], BF16, kind="ExternalInput")
    ropeBq = nc.dram_tensor("ropeBq", [DH, S], BF16, kind="ExternalInput")
    ropeAk = nc.dram_tensor("ropeAk", [DH, S], BF16, kind="ExternalInput")
    ropeBk = nc.dram_tensor("ropeBk", [DH, S], BF16, kind="ExternalInput")
    maskd = nc.dram_tensor("maskd", [128, 128], F32, kind="ExternalInput")
    identd = nc.dram_tensor("identd", [128, 128], BF16, kind="ExternalInput")
    out = nc.dram_tensor("out", [512, D], F32, kind="ExternalOutput")

    # packed P offsets: row i holds cols [128i, S) at packed offset
    poff = []
    o = 0
    for i in range(NI):
        poff.append(o)
        o += S - 128 * i
    PTOT = o  # 18432

    MTORD = (2, 3, 0, 1)   # k, v, q0, q1 — attention-critical first

    with tile.TileContext(nc) as tc:
        with tc.tile_pool(name="const", bufs=1) as cpool, \
             tc.tile_pool(name="pers", bufs=1) as pers, \
             tc.tile_pool(name="dram", bufs=1, space="DRAM") as dram:

            # ---- small constants on the scalar queue ----
            cw_sb = []
            for mt in range(4):
                t = cpool.tile([128, K_CONV], F32, tag=f"cw{mt}",
                               name=f"cw{mt}")
                nc.scalar.dma_start(t[:], cw.ap()[128 * mt:128 * mt + 128, :])
                cw_sb.append(t)
            mask_sb = cpool.tile([128, 128], F32, tag="mask")
            nc.scalar.dma_start(mask_sb[:], maskd.ap())
            ident_sb = cpool.tile([128, 128], BF16, tag="ident")
            nc.scalar.dma_start(ident_sb[:], identd.ap())
            ones_col_f = cpool.tile([128, 1], F32, tag="ocf")
            nc.vector.memset(ones_col_f[:], 1.0)
            ones_row_f = cpool.tile([1, 128], F32, tag="orf")
            nc.vector.memset(ones_row_f[:], 1.0)
            ones_row = cpool.tile([1, 128], F32R, tag="or")
            nc.scalar.copy(ones_row[:], ones_row_f[:])
            ones_col_bf = cpool.tile([128, 1], BF16, tag="ocb")
            nc.scalar.copy(ones_col_bf[:], ones_col_f[:])
            eps_sb = cpool.tile([1, 1], F32, tag="eps")
            nc.vector.memset(eps_sb[:], EPS)
            s0_sb = []
            for mt in range(4):
                t = cpool.tile([128, 1], F32, tag=f"s0{mt}", name=f"s0{mt}")
                nc.vector.tensor_scalar_add(t[:], cw_sb[mt][:, 0:1], 1.0)
                s0_sb.append(t)

            roped = {}
            vaug = {}
            rstdkT = {}
            for b in range(B):
                for mt in range(3):
                    roped[(b, mt)] = pers.tile([128, S], BF16,
                                               tag=f"roped{b}{mt}",
                                               name=f"roped{b}{mt}")
                vaug[b] = pers.tile([128, NI * 128], BF16, tag=f"vaug{b}",
                                    name=f"vaug{b}")
                rstdkT[b] = pers.tile([128, NI], F32, tag=f"rstdkT{b}",
                                      name=f"rstdkT{b}")

            srt_d = {b: dram.tile([NI, 128], BF16, tag=f"srtd{b}",
                                  name=f"srt_d{b}") for b in range(B)}
            a2a_in = [dram.tile([1024, 512], BF16, tag=f"a2ai{h}",
                                name=f"a2a_in{h}") for h in range(2)]
            a2a_out = [dram.tile([1024, 512], BF16, tag=f"a2ao{h}",
                                 name=f"a2a_out{h}") for h in range(2)]

            rope_cm = tc.tile_pool(name="ropes", bufs=1)
            rpool = rope_cm.__enter__()
            prep_cm = tc.tile_pool(name="prep", bufs=1)
            prep = prep_cm.__enter__()
            ropes = {}

            def qkv_batch(b, bw, cn, sq, hv):
                hsrc = (hsT.ap()[:, b * S:(b + 1) * S]
                        .rearrange("(k p) s -> p k s", p=128))
                for pc in range(4):
                    nc.scalar.dma_start(
                        hv[32 * pc:32 * (pc + 1), 0, :],
                        hsrc[32 * pc:32 * (pc + 1), 0, :])
                for k in range(1, 16):
                    nc.scalar.dma_start(hv[:, k, :], hsrc[:, k, :])
                if b == 0:
                    for nm, t in (("Ak", ropeAk), ("Bk", ropeBk),
                                  ("Aq", ropeAq), ("Bq", ropeBq)):
                        rt = rpool.tile([DH, S], BF16, tag=f"rope{nm}",
                                        name=f"rope{nm}")
                        nc.scalar.dma_start(rt[:], t.ap())
                        ropes[nm] = rt

                with tc.tile_pool(name=f"qps{b}", bufs=2,
                                  space="PSUM") as qps:
                    for mt in MTORD:
                        psums = [qps.tile([128, 512], F32, tag=f"q{n}",
                                          name=f"q{n}") for n in range(NCH)]
                        for k in range(16):
                            wt_k = prep.tile([128, 128], BF16, tag="wtk",
                                             bufs=8, name="wt_k")
                            nc.sync.dma_start(
                                wt_k[:],
                                wT.ap()[128 * k:128 * (k + 1),
                                        128 * mt:128 * (mt + 1)])
                            for n in range(NCH):
                                nc.tensor.matmul(
                                    psums[n][:], wt_k[:],
                                    hv[:, k, 512 * n:512 * (n + 1)],
                                    start=(k == 0), stop=(k == 15))
                        raw = bw.tile([128, S], BF16, tag=f"raw{b}",
                                      bufs=1, name="raw")
                        c = cn[mt]
                        for n in range(NCH):
                            sl = slice(512 * n, 512 * (n + 1))
                            nc.scalar.copy(raw[:, sl], psums[n][:])
                            nc.scalar.activation(c[:, sl], psums[n][:],
                                                 AF.Copy,
                                                 scale=s0_sb[mt][:])
                        for k in range(1, K_CONV):
                            nc.vector.scalar_tensor_tensor(
                                c[:, k:S], raw[:, 0:S - k],
                                cw_sb[mt][:, k:k + 1], c[:, k:S],
                                ALU.mult, ALU.add)
                        if mt != 3:
                            nc.vector.tensor_mul(sq[mt][:], c[:], c[:])

            def norm_rope_batch(b, cn, sq):
                va = vaug[b]
                with tc.tile_pool(name=f"nps{b}", bufs=2,
                                  space="PSUM") as nps, \
                     tc.tile_pool(name=f"bps{b}", bufs=2,
                                  space="PSUM") as bps, \
                     tc.tile_pool(name=f"tps{b}", bufs=2,
                                  space="PSUM") as tps:
                    for i in range(NI):
                        tp = tps.tile([128, 128], BF16, tag="tp",
                                      name="tp")
                        nc.tensor.transpose(tp[:],
                                            cn[3][:, 128 * i:128 * (i + 1)],
                                            ident_sb[:])
                        nc.scalar.copy(va[:, 128 * i:128 * (i + 1)], tp[:])
                    srtk = prep.tile([1, S], BF16, tag="srtk", name="srtk")
                    for mt in (2, 0, 1):
                        is_q = mt < 2
                        recq = []
                        for cch in range(NCH):
                            sl = slice(512 * cch, 512 * (cch + 1))
                            sp = nps.tile([1, 512], F32, tag="ssq")
                            nc.tensor.matmul(sp[:], ones_col_bf[:],
                                             sq[mt][:, sl],
                                             start=True, stop=True)
                            if is_q:
                                srt = prep.tile([1, 512], F32R, tag="srtq",
                                                bufs=2, name="srt")
                                nc.scalar.activation(
                                    srt[:], sp[:], AF.Sqrt,
                                    bias=eps_sb[:], scale=1.0 / DH)
                                bp = bps.tile([128, 512], F32, tag="bcp")
                                nc.tensor.matmul(bp[:], ones_row[:],
                                                 srt[:], start=True,
                                                 stop=True)
                                rq = prep.tile([128, 512], F32, tag="recq",
                                               bufs=4, name="rq")
                                nc.vector.reciprocal_approx_fast(rq[:],
                                                                 bp[:])
                                recq.append(rq)
                            else:
                                nc.scalar.activation(
                                    srtk[:, sl], sp[:], AF.Sqrt,
                                    bias=eps_sb[:], scale=1.0 / DH)
                        x = cn[mt]
                        A_ = ropes["Aq"] if is_q else ropes["Ak"]
                        B_ = ropes["Bq"] if is_q else ropes["Bk"]
                        sh = prep.tile([128, S], BF16, tag="sh", bufs=1,
                                       name="sh")
                        nc.gpsimd.dma_start(sh[0:64, :], x[64:128, :])
                        nc.gpsimd.dma_start(sh[64:128, :], x[0:64, :])
                        nc.vector.tensor_mul(sh[:], sh[:], B_[:])
                        tm = prep.tile([128, S], BF16, tag="tm", bufs=1,
                                       name="tm")
                        nc.vector.tensor_mul(tm[:], x[:], A_[:])
                        ro = roped[(b, mt)]
                        if is_q:
                            nc.vector.tensor_add(tm[:], tm[:], sh[:])
                            for cch in range(NCH):
                                sl = slice(512 * cch, 512 * (cch + 1))
                                nc.vector.tensor_mul(ro[:, sl], tm[:, sl],
                                                     recq[cch][:])
                        else:
                            nc.vector.tensor_add(ro[:], tm[:], sh[:])
                            nc.scalar.dma_start(srt_d[b][:], srtk[:])
                            srtkT = prep.tile([128, NI], BF16, tag="srtkT",
                                              name="srtkT")
                            nc.scalar.dma_start(
                                srtkT[:],
                                srt_d[b][:].rearrange("i p -> p i"))
                            nc.vector.reciprocal(rstdkT[b][:], srtkT[:])

            def attn_bh(h, b):
                KT = roped[(b, 2)]
                QT = roped[(b, h)]
                va = vaug[b]
                rkt = rstdkT[b]
                with tc.tile_pool(name=f"pp{h}{b}", bufs=1) as ppool:
                    pt = ppool.tile([128, PTOT], BF16, tag="pk", name="pk")

                    def pslice(i, j, off):
                        a = poff[i] + 512 * j + off - 128 * i
                        return pt[:, a:a + 512 - off]

                    with tc.tile_pool(name=f"sc{h}{b}", bufs=2,
                                      space="PSUM") as scps:
                        for i in range(NI):
                            lo = 128 * i
                            sc = scps.tile([128, 2048], F32, tag="sc",
                                           name="sc")
                            for n in range(lo // 512, NCH):
                                c0 = max(lo, 512 * n)
                                nc.tensor.matmul(
                                    sc[:, c0:512 * (n + 1)],
                                    KT[:, lo:lo + 128],
                                    QT[:, c0:512 * (n + 1)],
                                    start=True, stop=True)
                            nc.vector.tensor_add(
                                sc[:, lo:lo + 128],
                                sc[:, lo:lo + 128], mask_sb[:])
                            nc.scalar.activation(
                                pt[:, poff[i]:poff[i] + S - lo],
                                sc[:, lo:S], AF.Exp,
                                scale=rkt[:, i:i + 1])

                    with tc.tile_pool(name=f"pv{h}{b}", bufs=1,
                                      space="PSUM") as pvps, \
                         tc.tile_pool(name=f"st{h}{b}", bufs=2) as stp:
                        ots, dens = [], []
                        for j in range(NCH):
                            jmax = 4 * j + 3
                            ot = pvps.tile([128, 512], F32, tag=f"ot{j}",
                                           name=f"ot{j}")
                            for i in range(jmax + 1):
                                off = max(0, 128 * i - 512 * j)
                                nc.tensor.matmul(
                                    ot[:, off:512],
                                    va[:, 128 * i:128 * (i + 1)],
                                    pslice(i, j, off),
                                    start=(i == 0), stop=(i == jmax))
                            den = pvps.tile([1, 512], F32, tag="dn",
                                            bufs=2, name="den")
                            for i in range(jmax + 1):
                                off = max(0, 128 * i - 512 * j)
                                nc.tensor.matmul(
                                    den[:, off:512], ones_col_bf[:],
                                    pslice(i, j, off),
                                    start=(i == 0), stop=(i == jmax))
                            dsb = stp.tile([1, 512], F32R, tag="dsb",
                                           bufs=2, name="dsb")
                            nc.scalar.copy(dsb[:], den[:])
                            ots.append(ot)
                            dens.append(dsb)
                        for j in range(NCH):
                            bde = pvps.tile([128, 512], F32, tag="bd",
                                            bufs=2, name="bde")
                            nc.tensor.matmul(bde[:], ones_row[:],
                                             dens[j][:], start=True,
                                             stop=True)
                            brc = stp.tile([128, 512], F32, tag="brc",
                                           bufs=2, name="brc")
                            nc.vector.reciprocal_approx_fast(brc[:],
                                                             bde[:])
                            asb = stp.tile([128, 512], BF16, tag="asb",
                                           bufs=2, name="asb")
                            nc.vector.tensor_mul(asb[:], ots[j][:],
                                                 brc[:])
                            nc.sync.dma_start(
                                a2a_in[h][128 * (4 * b + j):
                                          128 * (4 * b + j + 1), :],
                                asb[:])

            # ================== phase schedule ==================
            # batch 0
            cn0_cm = tc.tile_pool(name="cn0", bufs=1)
            bw0 = cn0_cm.__enter__()
            cn0 = {mt: bw0.tile([128, S], BF16, tag=f"cn0{mt}",
                                name=f"cn0{mt}") for mt in range(4)}
            sq0_cm = tc.tile_pool(name="sq0", bufs=1)
            sw0 = sq0_cm.__enter__()
            sq0 = {mt: sw0.tile([128, S], BF16, tag=f"sq0{mt}",
                                name=f"sq0{mt}") for mt in range(3)}
            hs0_cm = tc.tile_pool(name="hs0", bufs=1)
            hs0_pool = hs0_cm.__enter__()
            hs0 = hs0_pool.tile([128, 16 * S], BF16, tag="hs0", name="hs0")
            qkv_batch(0, bw0, cn0, sq0,
                      hs0[:].rearrange("p (k s) -> p k s", s=S))
            hs0_cm.__exit__(None, None, None)
            norm_rope_batch(0, cn0, sq0)
            sq0_cm.__exit__(None, None, None)
            cn0_cm.__exit__(None, None, None)

            attn_bh(0, 0)
            attn_bh(1, 0)

            # batch 1
            cn1_cm = tc.tile_pool(name="cn1", bufs=1)
            bw1 = cn1_cm.__enter__()
            cn1 = {mt: bw1.tile([128, S], BF16, tag=f"cn1{mt}",
                                name=f"cn1{mt}") for mt in range(4)}
            sq1_cm = tc.tile_pool(name="sq1", bufs=1)
            sw1 = sq1_cm.__enter__()
            sq1 = {mt: sw1.tile([128, S], BF16, tag=f"sq1{mt}",
                                name=f"sq1{mt}") for mt in range(3)}
            hs1_cm = tc.tile_pool(name="hs1", bufs=1)
            hs1_pool = hs1_cm.__enter__()
            hs1 = hs1_pool.tile([128, 16 * S], BF16, tag="hs1", name="hs1")
            qkv_batch(1, bw1, cn1, sq1,
                      hs1[:].rearrange("p (k s) -> p k s", s=S))
            hs1_cm.__exit__(None, None, None)
            norm_rope_batch(1, cn1, sq1)
            sq1_cm.__exit__(None, None, None)
            cn1_cm.__exit__(None, None, None)
            prep_cm.__exit__(None, None, None)
            rope_cm.__exit__(None, None, None)

            # Wo resident + av staging
            wo_cm = tc.tile_pool(name="wo", bufs=1)
            wo_pool = wo_cm.__enter__()
            wo_sb = wo_pool.tile([128, 16 * D], BF16, tag="wo",
                                 name="wo_sb")
            wov = wo_sb[:].rearrange("p (k c) -> p k c", c=D)
            wsrc = woT.ap().rearrange("(k p) c -> p k c", p=128)
            for g in range(16):
                nc.sync.dma_start(wov[:, g, :], wsrc[:, g, :])
            opool_cm = tc.tile_pool(name="opool", bufs=1)
            opool = opool_cm.__enter__()
            av = []

            attn_bh(0, 1)
            nc.gpsimd.collective_compute(
                "AllToAll", ALU.bypass,
                replica_groups=[list(range(N_CORES))],
                ins=[a2a_in[0].opt()], outs=[a2a_out[0].opt()],
                cc_dim="Partition")
            at0 = opool.tile([128, 8 * 512], BF16, tag="av0", name="av0")
            nc.sync.dma_start(
                at0[:].rearrange("p (k s) -> p k s", s=512),
                a2a_out[0][:].rearrange("(k p) s -> p k s", p=128))
            av.append(at0[:].rearrange("p (k s) -> p k s", s=512))

            attn_bh(1, 1)
            nc.gpsimd.collective_compute(
                "AllToAll", ALU.bypass,
                replica_groups=[list(range(N_CORES))],
                ins=[a2a_in[1].opt()], outs=[a2a_out[1].opt()],
                cc_dim="Partition")
            at1 = opool.tile([128, 8 * 512], BF16, tag="av1", name="av1")
            nc.sync.dma_start(
                at1[:].rearrange("p (k s) -> p k s", s=512),
                a2a_out[1][:].rearrange("(k p) s -> p k s", p=128))
            av.append(at1[:].rearrange("p (k s) -> p k s", s=512))

            # ====================== out projection ====================
            with tc.tile_pool(name="ops", bufs=2, space="PSUM") as ops:
                for pair in ((0, 1), (2, 3)):
                    pso = {mp: [ops.tile([128, 512], F32,
                                         tag=f"po{pi * NCH + n}", bufs=1,
                                         name=f"po{pi}{n}")
                                for n in range(NCH)]
                           for pi, mp in enumerate(pair)}
                    for g in [0, 2, 4, 6, 8, 10, 12, 14,
                              1, 3, 5, 7, 9, 11, 13, 15]:
                        for mp in pair:
                            stat = av[g % 2][:, g // 2,
                                             128 * mp:128 * (mp + 1)]
                            for n in range(NCH):
                                nc.tensor.matmul(
                                    pso[mp][n][:], stat,
                                    wov[:, g, 512 * n:512 * (n + 1)],
                                    start=(g == 0), stop=(g == 15))
                    for mp in pair:
                        for n in range(NCH):
                            os_t = opool.tile([128, 512], F32, tag="osb",
                                              bufs=4, name="os_t")
                            nc.scalar.copy(os_t[:], pso[mp][n][:])
                            nc.sync.dma_start(
                                out.ap()[128 * mp:128 * (mp + 1),
                                         512 * n:512 * (n + 1)], os_t[:])
            opool_cm.__exit__(None, None, None)
            wo_cm.__exit__(None, None, None)

    nc.compile()
    return nc


_NC_CACHE = None


def _get_nc():
    global _NC_CACHE
    if _NC_CACHE is None:
        _NC_CACHE = _build()
    return _NC_CACHE


def _host_prep(inputs):
    hs = np.asarray(inputs["hidden_states"], dtype=np.float32)
    Wq = np.asarray(inputs["Wq"], dtype=np.float32)
    Wk = np.asarray(inputs["Wk"], dtype=np.float32)
    Wv = np.asarray(inputs["Wv"], dtype=np.float32)
    Wo = np.asarray(inputs["Wo"], dtype=np.float32)
    cqw = np.asarray(inputs["canon_q_w"], dtype=np.float32)
    ckw = np.asarray(inputs["canon_k_w"], dtype=np.float32)
    cvw = np.asarray(inputs["canon_v_w"], dtype=np.float32)
    qnw = np.asarray(inputs["q_norm_w"], dtype=np.float32)
    knw = np.asarray(inputs["k_norm_w"], dtype=np.float32)

    bf = ml_dtypes.bfloat16
    hsT = np.ascontiguousarray(
        np.concatenate([hs[0].T, hs[1].T], axis=1)).astype(bf)
    WqT, WkT, WvT = Wq.T, Wk.T, Wv.T
    woT = np.ascontiguousarray(Wo.T).astype(bf)

    inv_freq = 1.0 / (10000.0 ** (np.arange(0, DH, 2, dtype=np.float64) / DH))
    freqs = np.arange(S, dtype=np.float64)[:, None] * inv_freq
    emb = np.concatenate([freqs, freqs], axis=-1)
    cosT, sinT = np.cos(emb).T, np.sin(emb).T

    def make_rope(normw, scale):
        A = cosT * normw[:, None] * scale
        wswap = normw[(np.arange(DH) + 64) % DH]
        sign = np.where(np.arange(DH) < 64, -1.0, 1.0)
        Bc = sinT * wswap[:, None] * sign[:, None] * scale
        return (np.ascontiguousarray(A).astype(bf),
                np.ascontiguousarray(Bc).astype(bf))

    Aq, Bq = make_rope(qnw, SCALE)
    Ak, Bk = make_rope(knw, 1.0)

    p = np.arange(128)[:, None]
    f = np.arange(128)[None, :]
    maskd = np.where(p <= f, 0.0, NEG).astype(np.float32)
    identd = np.eye(128, dtype=np.float32).astype(bf)

    in_maps = []
    for r in range(N_CORES):
        wTc = np.ascontiguousarray(np.concatenate(
            [WqT[:, 256 * r:256 * r + 256],
             WkT[:, 128 * r:128 * r + 128],
             WvT[:, 128 * r:128 * r + 128]], axis=1)).astype(bf)
        cwc = np.ascontiguousarray(np.concatenate(
            [cqw[256 * r:256 * r + 256],
             ckw[128 * r:128 * r + 128],
             cvw[128 * r:128 * r + 128]], axis=0)).astype(np.float32)
        in_maps.append({
            "hsT": hsT, "wT": wTc, "woT": woT, "cw": cwc,
            "ropeAq": Aq, "ropeBq": Bq, "ropeAk": Ak, "ropeBk": Bk,
            "maskd": maskd, "identd": identd,
        })
    return in_maps


def kernel(**inputs):
    nc = _get_nc()
    in_maps = _host_prep(inputs)
    res = run_bass_kernel_spmd(nc, in_maps, core_ids=list(range(N_CORES)))
    full = np.empty((B, S, D), np.float32)
    for r in range(N_CORES):
        full[r // 4, 512 * (r % 4):512 * (r % 4 + 1), :] = res.results[r]["out"]
    return full


# revision 30
# speedup vs baseline: 1.0110x; 1.0110x over previous
"""Trainium2 Bass kernel for CanonCausalMultiheadAttn (v3).

Sharding: tensor-parallel over heads across 8 cores (2 q-heads + 1 kv-head
per core), both batches replicated. Attention outputs are exchanged with
TWO AllToAlls (one per local q-head) so round 0 overlaps round-1 compute;
each core then owns one (batch, 512-seq-slice) of the output projection.

Per-core pipeline (B=2, S=2048, D=2048 hardcoded):
  QKV proj: bf16 matmuls, the PE queue is pure matmuls plus tiny norm
  reductions -> canon conv on full bf16 rows (DVE taps at 2x 16-bit rate,
  first tap folded into the ACT psum-drain copy) -> qk rmsnorm:
  sum-of-squares via ones-column matmuls, q-rstd broadcast via ones-row
  matmul + wide 128-lane reciprocal (no single-partition reciprocals
  anywhere), k-rstd transposed via DRAM and applied as the EXP
  per-partition scale -> RoPE (bf16 DVE, norm weight & 1/sqrt(dh) folded
  into bf16 tables) -> causal attention with scores in [Sk, Sq] layout;
  per K-block one wide matmul row + one wide EXP -> PV swapped
  (stationary = V block, moving = P) emitting O'^T [dh, q] directly in
  a2a layout, denominators via ones-column rowsum matmuls, normalization
  via broadcast matmul + wide reciprocal folded into the psum drain ->
  AllToAll x2 -> output projection (stationary = received attn block,
  resident Wo in SBUF). DMA triggers are spread across engine queues
  (weights/activations on Sync, V transposes on GpSimd, rope shift on
  Vector, norm roundtrip + tables on Scalar) to avoid head-of-line
  blocking of the weight stream.
"""
import sys

sys.path.insert(0, '/opt/trn_rl_repo')

import numpy as np
import ml_dtypes

import concourse.bass as bass
import concourse.mybir as mybir
import concourse.tile as tile
from concourse import bacc
from concourse.bass_utils import run_bass_kernel_spmd

F32 = mybir.dt.float32
F32R = mybir.dt.float32r
BF16 = mybir.dt.bfloat16
AF = mybir.ActivationFunctionType
ALU = mybir.AluOpType

B, S, D = 2, 2048, 2048
NH, NKV, DH = 16, 8, 128
K_CONV = 4
EPS = 1e-6
SCALE = 1.0 / float(np.sqrt(DH))
NEG = -1e9
N_CORES = 8
NCH = S // 512          # 512-wide seq chunks
NI = S // 128           # 128-wide Sk blocks


def _build():
    nc = bacc.Bacc("TRN2", target_bir_lowering=False, debug=False,
                   num_devices=N_CORES)

    hsT = nc.dram_tensor("hsT", [D, B * S], BF16, kind="ExternalInput")
    wT = nc.dram_tensor("wT", [D, 512], BF16, kind="ExternalInput")
    woT = nc.dram_tensor("woT", [D, D], BF16, kind="ExternalInput")
    cw = nc.dram_tensor("cw", [512, K_CONV], F32, kind="ExternalInput")
    ropeAq = nc.dram_tensor("ropeAq", [DH, S], BF16, kind="ExternalInput")
    ropeBq = nc.dram_tensor("ropeBq", [DH, S], BF16, kind="ExternalInput")
    ropeAk = nc.dram_tensor("ropeAk", [DH, S], BF16, kind="ExternalInput")
    ropeBk = nc.dram_tensor("ropeBk", [DH, S], BF16, kind="ExternalInput")
    maskd = nc.dram_tensor("maskd", [128, 128], F32, kind="ExternalInput")
    identd = nc.dram_tensor("identd", [128, 128], BF16, kind="ExternalInput")
    out = nc.dram_tensor("out", [512, D], F32, kind="ExternalOutput")

    # packed P offsets: row i holds cols [128i, S) at packed offset
    poff = []
    o = 0
    for i in range(NI):
        poff.append(o)
        o += S - 128 * i
    PTOT = o  # 18432

    MTORD = (0, 1, 2, 3)

    with tile.TileContext(nc) as tc:
        with tc.tile_pool(name="const", bufs=1) as cpool, \
             tc.tile_pool(name="pers", bufs=1) as pers, \
             tc.tile_pool(name="dram", bufs=1, space="DRAM") as dram:

            # ---- small constants on the scalar queue ----
            cw_sb = []
            for mt in range(4):
                t = cpool.tile([128, K_CONV], F32, tag=f"cw{mt}",
                               name=f"cw{mt}")
                nc.scalar.dma_start(t[:], cw.ap()[128 * mt:128 * mt + 128, :])
                cw_sb.append(t)
            mask_sb = cpool.tile([128, 128], F32, tag="mask")
            nc.scalar.dma_start(mask_sb[:], maskd.ap())
            ident_sb = cpool.tile([128, 128], BF16, tag="ident")
            nc.scalar.dma_start(ident_sb[:], identd.ap())
            ones_col_f = cpool.tile([128, 1], F32, tag="ocf")
            nc.vector.memset(ones_col_f[:], 1.0)
            ones_row_f = cpool.tile([1, 128], F32, tag="orf")
            nc.vector.memset(ones_row_f[:], 1.0)
            ones_row = cpool.tile([1, 128], F32R, tag="or")
            nc.scalar.copy(ones_row[:], ones_row_f[:])
            ones_col_bf = cpool.tile([128, 1], BF16, tag="ocb")
            nc.scalar.copy(ones_col_bf[:], ones_col_f[:])
            eps_sb = cpool.tile([1, 1], F32, tag="eps")
            nc.vector.memset(eps_sb[:], EPS)
            s0_sb = []
            for mt in range(4):
                t = cpool.tile([128, 1], F32, tag=f"s0{mt}", name=f"s0{mt}")
                nc.vector.tensor_scalar_add(t[:], cw_sb[mt][:, 0:1], 1.0)
                s0_sb.append(t)

            roped = {}
            vaug = {}
            rstdkT = {}
            for b in range(B):
                for mt in range(3):
                    roped[(b, mt)] = pers.tile([128, S], BF16,
                                               tag=f"roped{b}{mt}",
                                               name=f"roped{b}{mt}")
                vaug[b] = pers.tile([128, NI * 128], BF16, tag=f"vaug{b}",
                                    name=f"vaug{b}")
                rstdkT[b] = pers.tile([128, NI], F32, tag=f"rstdkT{b}",
                                      name=f"rstdkT{b}")

            srt_d = {b: dram.tile([NI, 128], BF16, tag=f"srtd{b}",
                                  name=f"srt_d{b}") for b in range(B)}
            a2a_in = [dram.tile([1024, 512], BF16, tag=f"a2ai{h}",
                                name=f"a2a_in{h}") for h in range(2)]
            a2a_out = [dram.tile([1024, 512], BF16, tag=f"a2ao{h}",
                                 name=f"a2a_out{h}") for h in range(2)]

            rope_cm = tc.tile_pool(name="ropes", bufs=1)
            rpool = rope_cm.__enter__()
            prep_cm = tc.tile_pool(name="prep", bufs=1)
            prep = prep_cm.__enter__()
            ropes = {}

            def qkv_batch(b, bw, cn, sq, hv):
                hsrc = (hsT.ap()[:, b * S:(b + 1) * S]
                        .rearrange("(k p) s -> p k s", p=128))
                for pc in range(4):
                    nc.scalar.dma_start(
                        hv[32 * pc:32 * (pc + 1), 0, :],
                        hsrc[32 * pc:32 * (pc + 1), 0, :])
                for k in range(1, 16):
                    nc.scalar.dma_start(hv[:, k, :], hsrc[:, k, :])
                if b == 0:
                    for nm, t in (("Ak", ropeAk), ("Bk", ropeBk),
                                  ("Aq", ropeAq), ("Bq", ropeBq)):
                        rt = rpool.tile([DH, S], BF16, tag=f"rope{nm}",
                                        name=f"rope{nm}")
                        nc.scalar.dma_start(rt[:], t.ap())
                        ropes[nm] = rt

                with tc.tile_pool(name=f"qps{b}", bufs=2,
                                  space="PSUM") as qps:
                    for mt in MTORD:
                        psums = [qps.tile([128, 512], F32, tag=f"q{n}",
                                          name=f"q{n}") for n in range(NCH)]
                        for k in range(16):
                            wt_k = prep.tile([128, 128], BF16, tag="wtk",
                                             bufs=8, name="wt_k")
                            nc.sync.dma_start(
                                wt_k[:],
                                wT.ap()[128 * k:128 * (k + 1),
                                        128 * mt:128 * (mt + 1)])
                            for n in range(NCH):
                                nc.tensor.matmul(
                                    psums[n][:], wt_k[:],
                                    hv[:, k, 512 * n:512 * (n + 1)],
                                    start=(k == 0), stop=(k == 15))
                        raw = bw.tile([128, S], BF16, tag=f"raw{b}",
                                      bufs=1, name="raw")
                        c = cn[mt]
                        for n in range(NCH):
                            sl = slice(512 * n, 512 * (n + 1))
                            nc.scalar.copy(raw[:, sl], psums[n][:])
                            nc.scalar.activation(c[:, sl], psums[n][:],
                                                 AF.Copy,
                                                 scale=s0_sb[mt][:])
                        for k in range(1, K_CONV):
                            nc.vector.scalar_tensor_tensor(
                                c[:, k:S], raw[:, 0:S - k],
                                cw_sb[mt][:, k:k + 1], c[:, k:S],
                                ALU.mult, ALU.add)
                        if mt != 3:
                            nc.vector.tensor_mul(sq[mt][:], c[:], c[:])

            def norm_rope_batch(b, cn, sq):
                va = vaug[b]
                with tc.tile_pool(name=f"nps{b}", bufs=2,
                                  space="PSUM") as nps, \
                     tc.tile_pool(name=f"bps{b}", bufs=2,
                                  space="PSUM") as bps, \
                     tc.tile_pool(name=f"tps{b}", bufs=2,
                                  space="PSUM") as tps:
                    for i in range(NI):
                        tp = tps.tile([128, 128], BF16, tag="tp",
                                      name="tp")
                        nc.tensor.transpose(tp[:],
                                            cn[3][:, 128 * i:128 * (i + 1)],
                                            ident_sb[:])
                        nc.scalar.copy(va[:, 128 * i:128 * (i + 1)], tp[:])
                    srtk = prep.tile([1, S], BF16, tag="srtk", name="srtk")
                    for mt in (2, 0, 1):
                        is_q = mt < 2
                        recq = []
                        for cch in range(NCH):
                            sl = slice(512 * cch, 512 * (cch + 1))
                            sp = nps.tile([1, 512], F32, tag="ssq")
                            nc.tensor.matmul(sp[:], ones_col_bf[:],
                                             sq[mt][:, sl],
                                             start=True, stop=True)
                            if is_q:
                                srt = prep.tile([1, 512], F32R, tag="srtq",
                                                bufs=2, name="srt")
                                nc.scalar.activation(
                                    srt[:], sp[:], AF.Sqrt,
                                    bias=eps_sb[:], scale=1.0 / DH)
                                bp = bps.tile([128, 512], F32, tag="bcp")
                                nc.tensor.matmul(bp[:], ones_row[:],
                                                 srt[:], start=True,
                                                 stop=True)
                                rq = prep.tile([128, 512], F32, tag="recq",
                                               bufs=4, name="rq")
                                nc.vector.reciprocal_approx_fast(rq[:],
                                                                 bp[:])
                                recq.append(rq)
                            else:
                                nc.scalar.activation(
                                    srtk[:, sl], sp[:], AF.Sqrt,
                                    bias=eps_sb[:], scale=1.0 / DH)
                        x = cn[mt]
                        A_ = ropes["Aq"] if is_q else ropes["Ak"]
                        B_ = ropes["Bq"] if is_q else ropes["Bk"]
                        sh = prep.tile([128, S], BF16, tag="sh", bufs=1,
                                       name="sh")
                        nc.gpsimd.dma_start(sh[0:64, :], x[64:128, :])
                        nc.gpsimd.dma_start(sh[64:128, :], x[0:64, :])
                        nc.vector.tensor_mul(sh[:], sh[:], B_[:])
                        tm = prep.tile([128, S], BF16, tag="tm", bufs=1,
                                       name="tm")
                        nc.vector.tensor_mul(tm[:], x[:], A_[:])
                        ro = roped[(b, mt)]
                        if is_q:
                            nc.vector.tensor_add(tm[:], tm[:], sh[:])
                            for cch in range(NCH):
                                sl = slice(512 * cch, 512 * (cch + 1))
                                nc.vector.tensor_mul(ro[:, sl], tm[:, sl],
                                                     recq[cch][:])
                        else:
                            nc.vector.tensor_add(ro[:], tm[:], sh[:])
                            nc.scalar.dma_start(srt_d[b][:], srtk[:])
                            srtkT = prep.tile([128, NI], BF16, tag="srtkT",
                                              name="srtkT")
                            nc.scalar.dma_start(
                                srtkT[:],
                                srt_d[b][:].rearrange("i p -> p i"))
                            nc.vector.reciprocal(rstdkT[b][:], srtkT[:])

            def attn_bh(h, b):
                KT = roped[(b, 2)]
                QT = roped[(b, h)]
                va = vaug[b]
                rkt = rstdkT[b]
                with tc.tile_pool(name=f"pp{h}{b}", bufs=1) as ppool, \
                     tc.tile_pool(name=f"sc{h}{b}", bufs=1,
                                  space="PSUM") as scps, \
                     tc.tile_pool(name=f"pv{h}{b}", bufs=1,
                                  space="PSUM") as pvps, \
                     tc.tile_pool(name=f"st{h}{b}", bufs=2) as stp:
                    pt = ppool.tile([128, PTOT], BF16, tag="pk", name="pk")

                    def pslice(i, j, off):
                        a = poff[i] + 512 * j + off - 128 * i
                        return pt[:, a:a + 512 - off]

                    ots, dsbs = {}, {}

                    def bmms(j):
                        jmax = 4 * j + 3
                        ot = pvps.tile([128, 512], F32, tag="ot", bufs=2,
                                       name="ot")
                        for i in range(jmax + 1):
                            off = max(0, 128 * i - 512 * j)
                            nc.tensor.matmul(
                                ot[:, off:512],
                                va[:, 128 * i:128 * (i + 1)],
                                pslice(i, j, off),
                                start=(i == 0), stop=(i == jmax))
                        den = pvps.tile([1, 512], F32, tag="dn", bufs=1,
                                        name="den")
                        for i in range(jmax + 1):
                            off = max(0, 128 * i - 512 * j)
                            nc.tensor.matmul(
                                den[:, off:512], ones_col_bf[:],
                                pslice(i, j, off),
                                start=(i == 0), stop=(i == jmax))
                        dsb = stp.tile([1, 512], F32R, tag="dsb", bufs=2,
                                       name="dsb")
                        nc.scalar.copy(dsb[:], den[:])
                        ots[j] = ot
                        dsbs[j] = dsb

                    def ep(j):
                        bde = pvps.tile([128, 512], F32, tag="bd", bufs=1,
                                        name="bde")
                        nc.tensor.matmul(bde[:], ones_row[:], dsbs[j][:],
                                         start=True, stop=True)
                        brc = stp.tile([128, 512], F32, tag="brc", bufs=2,
                                       name="brc")
                        nc.vector.reciprocal_approx_fast(brc[:], bde[:])
                        asb = stp.tile([128, 512], BF16, tag="asb",
                                       bufs=2, name="asb")
                        nc.vector.tensor_mul(asb[:], ots[j][:], brc[:])
                        nc.sync.dma_start(
                            a2a_in[h][128 * (4 * b + j):
                                      128 * (4 * b + j + 1), :],
                            asb[:])

                    for g in range(NCH):
                        for i in range(4 * g, 4 * g + 4):
                            lo = 128 * i
                            sc = scps.tile([128, 2048], F32, tag="sc",
                                           name="sc")
                            for n in range(lo // 512, NCH):
                                c0 = max(lo, 512 * n)
                                nc.tensor.matmul(
                                    sc[:, c0:512 * (n + 1)],
                                    KT[:, lo:lo + 128],
                                    QT[:, c0:512 * (n + 1)],
                                    start=True, stop=True)
                            nc.vector.tensor_add(
                                sc[:, lo:lo + 128],
                                sc[:, lo:lo + 128], mask_sb[:])
                            nc.scalar.activation(
                                pt[:, poff[i]:poff[i] + S - lo],
                                sc[:, lo:S], AF.Exp,
                                scale=rkt[:, i:i + 1])
                        if g > 0:
                            ep(g - 1)
                        bmms(g)
                    ep(NCH - 1)

            # ================== phase schedule ==================
            # batch 0
            cn0_cm = tc.tile_pool(name="cn0", bufs=1)
            bw0 = cn0_cm.__enter__()
            cn0 = {mt: bw0.tile([128, S], BF16, tag=f"cn0{mt}",
                                name=f"cn0{mt}") for mt in range(4)}
            sq0_cm = tc.tile_pool(name="sq0", bufs=1)
            sw0 = sq0_cm.__enter__()
            sq0 = {mt: sw0.tile([128, S], BF16, tag=f"sq0{mt}",
                                name=f"sq0{mt}") for mt in range(3)}
            hs0_cm = tc.tile_pool(name="hs0", bufs=1)
            hs0_pool = hs0_cm.__enter__()
            hs0 = hs0_pool.tile([128, 16 * S], BF16, tag="hs0", name="hs0")
            qkv_batch(0, bw0, cn0, sq0,
                      hs0[:].rearrange("p (k s) -> p k s", s=S))
            hs0_cm.__exit__(None, None, None)
            norm_rope_batch(0, cn0, sq0)
            sq0_cm.__exit__(None, None, None)
            cn0_cm.__exit__(None, None, None)

            attn_bh(0, 0)
            attn_bh(1, 0)

            # batch 1
            cn1_cm = tc.tile_pool(name="cn1", bufs=1)
            bw1 = cn1_cm.__enter__()
            cn1 = {mt: bw1.tile([128, S], BF16, tag=f"cn1{mt}",
                                name=f"cn1{mt}") for mt in range(4)}
            sq1_cm = tc.tile_pool(name="sq1", bufs=1)
            sw1 = sq1_cm.__enter__()
            sq1 = {mt: sw1.tile([128, S], BF16, tag=f"sq1{mt}",
                                name=f"sq1{mt}") for mt in range(3)}
            hs1_cm = tc.tile_pool(name="hs1", bufs=1)
            hs1_pool = hs1_cm.__enter__()
            hs1 = hs1_pool.tile([128, 16 * S], BF16, tag="hs1", name="hs1")
            qkv_batch(1, bw1, cn1, sq1,
                      hs1[:].rearrange("p (k s) -> p k s", s=S))
            hs1_cm.__exit__(None, None, None)
            norm_rope_batch(1, cn1, sq1)
            sq1_cm.__exit__(None, None, None)
            cn1_cm.__exit__(None, None, None)
            prep_cm.__exit__(None, None, None)
            rope_cm.__exit__(None, None, None)

            # Wo resident + av staging
            wo_cm = tc.tile_pool(name="wo", bufs=1)
            wo_pool = wo_cm.__enter__()
            wo_sb = wo_pool.tile([128, 16 * D], BF16, tag="wo",
                                 name="wo_sb")
            wov = wo_sb[:].rearrange("p (k c) -> p k c", c=D)
            wsrc = woT.ap().rearrange("(k p) c -> p k c", p=128)
            for g in range(16):
                nc.sync.dma_start(wov[:, g, :], wsrc[:, g, :])
            opool_cm = tc.tile_pool(name="opool", bufs=1)
            opool = opool_cm.__enter__()
            av = []

            attn_bh(0, 1)
            nc.gpsimd.collective_compute(
                "AllToAll", ALU.bypass,
                replica_groups=[list(range(N_CORES))],
                ins=[a2a_in[0].opt()], outs=[a2a_out[0].opt()],
                cc_dim="Partition")
            at0 = opool.tile([128, 8 * 512], BF16, tag="av0", name="av0")
            nc.sync.dma_start(
                at0[:].rearrange("p (k s) -> p k s", s=512),
                a2a_out[0][:].rearrange("(k p) s -> p k s", p=128))
            av.append(at0[:].rearrange("p (k s) -> p k s", s=512))

            attn_bh(1, 1)
            nc.gpsimd.collective_compute(
                "AllToAll", ALU.bypass,
                replica_groups=[list(range(N_CORES))],
                ins=[a2a_in[1].opt()], outs=[a2a_out[1].opt()],
                cc_dim="Partition")
            at1 = opool.tile([128, 8 * 512], BF16, tag="av1", name="av1")
            nc.sync.dma_start(
                at1[:].rearrange("p (k s) -> p k s", s=512),
                a2a_out[1][:].rearrange("(k p) s -> p k s", p=128))
            av.append(at1[:].rearrange("p (k s) -> p k s", s=512))

            # ====================== out projection ====================
            with tc.tile_pool(name="ops", bufs=2, space="PSUM") as ops:
                for pair in ((0, 1), (2, 3)):
                    pso = {mp: [ops.tile([128, 512], F32,
                                         tag=f"po{pi * NCH + n}", bufs=1,
                                         name=f"po{pi}{n}")
                                for n in range(NCH)]
                           for pi, mp in enumerate(pair)}
                    for g in [0, 2, 4, 6, 8, 10, 12, 14,
                              1, 3, 5, 7, 9, 11, 13, 15]:
                        for mp in pair:
                            stat = av[g % 2][:, g // 2,
                                             128 * mp:128 * (mp + 1)]
                            for n in range(NCH):
                                nc.tensor.matmul(
                                    pso[mp][n][:], stat,
                                    wov[:, g, 512 * n:512 * (n + 1)],
                                    start=(g == 0), stop=(g == 15))
                    for mp in pair:
                        for n in range(NCH):
                            os_t = opool.tile([128, 512], F32, tag="osb",
                                              bufs=4, name="os_t")
                            nc.scalar.copy(os_t[:], pso[mp][n][:])
                            nc.sync.dma_start(
                                out.ap()[128 * mp:128 * (mp + 1),
                                         512 * n:512 * (n + 1)], os_t[:])
            opool_cm.__exit__(None, None, None)
            wo_cm.__exit__(None, None, None)

    nc.compile()
    return nc


_NC_CACHE = None


def _get_nc():
    global _NC_CACHE
    if _NC_CACHE is None:
        _NC_CACHE = _build()
    return _NC_CACHE


def _host_prep(inputs):
    hs = np.asarray(inputs["hidden_states"], dtype=np.float32)
    Wq = np.asarray(inputs["Wq"], dtype=np.float32)
    Wk = np.asarray(inputs["Wk"], dtype=np.float32)
    Wv = np.asarray(inputs["Wv"], dtype=np.float32)
    Wo = np.asarray(inputs["Wo"], dtype=np.float32)
    cqw = np.asarray(inputs["canon_q_w"], dtype=np.float32)
    ckw = np.asarray(inputs["canon_k_w"], dtype=np.float32)
    cvw = np.asarray(inputs["canon_v_w"], dtype=np.float32)
    qnw = np.asarray(inputs["q_norm_w"], dtype=np.float32)
    knw = np.asarray(inputs["k_norm_w"], dtype=np.float32)

    bf = ml_dtypes.bfloat16
    hsT = np.ascontiguousarray(
        np.concatenate([hs[0].T, hs[1].T], axis=1)).astype(bf)
    WqT, WkT, WvT = Wq.T, Wk.T, Wv.T
    woT = np.ascontiguousarray(Wo.T).astype(bf)

    inv_freq = 1.0 / (10000.0 ** (np.arange(0, DH, 2, dtype=np.float64) / DH))
    freqs = np.arange(S, dtype=np.float64)[:, None] * inv_freq
    emb = np.concatenate([freqs, freqs], axis=-1)
    cosT, sinT = np.cos(emb).T, np.sin(emb).T

    def make_rope(normw, scale):
        A = cosT * normw[:, None] * scale
        wswap = normw[(np.arange(DH) + 64) % DH]
        sign = np.where(np.arange(DH) < 64, -1.0, 1.0)
        Bc = sinT * wswap[:, None] * sign[:, None] * scale
        return (np.ascontiguousarray(A).astype(bf),
                np.ascontiguousarray(Bc).astype(bf))

    Aq, Bq = make_rope(qnw, SCALE)
    Ak, Bk = make_rope(knw, 1.0)

    p = np.arange(128)[:, None]
    f = np.arange(128)[None, :]
    maskd = np.where(p <= f, 0.0, NEG).astype(np.float32)
    identd = np.eye(128, dtype=np.float32).astype(bf)

    in_maps = []
    for r in range(N_CORES):
        wTc = np.ascontiguousarray(np.concatenate(
            [WqT[:, 256 * r:256 * r + 256],
             WkT[:, 128 * r:128 * r + 128],
             WvT[:, 128 * r:128 * r + 128]], axis=1)).astype(bf)
        cwc = np.ascontiguousarray(np.concatenate(
            [cqw[256 * r:256 * r + 256],
             ckw[128 * r:128 * r + 128],
             cvw[128 * r:128 * r + 128]], axis=0)).astype(np.float32)
        in_maps.append({
            "hsT": hsT, "wT": wTc, "woT": woT, "cw": cwc,
            "ropeAq": Aq, "ropeBq": Bq, "ropeAk": Ak, "ropeBk": Bk,
            "maskd": maskd, "identd": identd,
        })
    return in_maps


def kernel(**inputs):
    nc = _get_nc()
    in_maps = _host_prep(inputs)
    res = run_bass_kernel_spmd(nc, in_maps, core_ids=list(range(N_CORES)))
    full = np.empty((B, S, D), np.float32)
    for r in range(N_CORES):
        full[r // 4, 512 * (r % 4):512 * (r % 4 + 1), :] = res.results[r]["out"]
    return full


# revision 31
# speedup vs baseline: 1.0809x; 1.0692x over previous
"""Trainium2 Bass kernel for CanonCausalMultiheadAttn (v3).

Sharding: tensor-parallel over heads across 8 cores (2 q-heads + 1 kv-head
per core), both batches replicated. Attention outputs are exchanged with
TWO AllToAlls (one per local q-head) so round 0 overlaps round-1 compute;
each core then owns one (batch, 512-seq-slice) of the output projection.

Per-core pipeline (B=2, S=2048, D=2048 hardcoded):
  QKV proj: bf16 matmuls, the PE queue is pure matmuls plus tiny norm
  reductions -> canon conv on full bf16 rows (DVE taps at 2x 16-bit rate,
  first tap folded into the ACT psum-drain copy) -> qk rmsnorm:
  sum-of-squares via ones-column matmuls, q-rstd broadcast via ones-row
  matmul + wide 128-lane reciprocal (no single-partition reciprocals
  anywhere), k-rstd transposed via DRAM and applied as the EXP
  per-partition scale -> RoPE (bf16 DVE, norm weight & 1/sqrt(dh) folded
  into bf16 tables) -> causal attention with scores in [Sk, Sq] layout;
  per K-block one wide matmul row + one wide EXP -> PV swapped
  (stationary = V block, moving = P) emitting O'^T [dh, q] directly in
  a2a layout, denominators via ones-column rowsum matmuls, normalization
  via broadcast matmul + wide reciprocal folded into the psum drain ->
  AllToAll x2 -> output projection (stationary = received attn block,
  resident Wo in SBUF). DMA triggers are spread across engine queues
  (weights/activations on Sync, V transposes on GpSimd, rope shift on
  Vector, norm roundtrip + tables on Scalar) to avoid head-of-line
  blocking of the weight stream.
"""
import sys

sys.path.insert(0, '/opt/trn_rl_repo')

import numpy as np
import ml_dtypes

import concourse.bass as bass
import concourse.mybir as mybir
import concourse.tile as tile
from concourse import bacc
from concourse.bass_utils import run_bass_kernel_spmd

F32 = mybir.dt.float32
F32R = mybir.dt.float32r
BF16 = mybir.dt.bfloat16
AF = mybir.ActivationFunctionType
ALU = mybir.AluOpType

B, S, D = 2, 2048, 2048
NH, NKV, DH = 16, 8, 128
K_CONV = 4
EPS = 1e-6
SCALE = 1.0 / float(np.sqrt(DH))
NEG = -1e9
N_CORES = 8
NCH = S // 512          # 512-wide seq chunks
NI = S // 128           # 128-wide Sk blocks


def _build():
    nc = bacc.Bacc("TRN2", target_bir_lowering=False, debug=False,
                   num_devices=N_CORES)

    hsT = nc.dram_tensor("hsT", [D, B * S], BF16, kind="ExternalInput")
    wT = nc.dram_tensor("wT", [D, 512], BF16, kind="ExternalInput")
    woT = nc.dram_tensor("woT", [D, D], BF16, kind="ExternalInput")
    cw = nc.dram_tensor("cw", [512, K_CONV], F32, kind="ExternalInput")
    ropeAq = nc.dram_tensor("ropeAq", [DH, S], BF16, kind="ExternalInput")
    ropeBq = nc.dram_tensor("ropeBq", [DH, S], BF16, kind="ExternalInput")
    ropeAk = nc.dram_tensor("ropeAk", [DH, S], BF16, kind="ExternalInput")
    ropeBk = nc.dram_tensor("ropeBk", [DH, S], BF16, kind="ExternalInput")
    maskd = nc.dram_tensor("maskd", [128, 128], F32, kind="ExternalInput")
    identd = nc.dram_tensor("identd", [128, 128], BF16, kind="ExternalInput")
    out = nc.dram_tensor("out", [512, D], F32, kind="ExternalOutput")

    # packed P offsets: row i holds cols [128i, S) at packed offset
    poff = []
    o = 0
    for i in range(NI):
        poff.append(o)
        o += S - 128 * i
    PTOT = o  # 18432

    MTORD = (2, 3, 0, 1)   # k, v, q0, q1 - attention-critical first

    with tile.TileContext(nc) as tc:
        with tc.tile_pool(name="const", bufs=1) as cpool, \
             tc.tile_pool(name="pers", bufs=1) as pers, \
             tc.tile_pool(name="dram", bufs=1, space="DRAM") as dram:

            # ---- small constants on the scalar queue ----
            cw_sb = []
            for mt in range(4):
                t = cpool.tile([128, K_CONV], F32, tag=f"cw{mt}",
                               name=f"cw{mt}")
                nc.scalar.dma_start(t[:], cw.ap()[128 * mt:128 * mt + 128, :])
                cw_sb.append(t)
            mask_sb = cpool.tile([128, 128], F32, tag="mask")
            nc.scalar.dma_start(mask_sb[:], maskd.ap())
            ident_sb = cpool.tile([128, 128], BF16, tag="ident")
            nc.scalar.dma_start(ident_sb[:], identd.ap())
            ones_col_f = cpool.tile([128, 1], F32, tag="ocf")
            nc.vector.memset(ones_col_f[:], 1.0)
            ones_row_f = cpool.tile([1, 128], F32, tag="orf")
            nc.vector.memset(ones_row_f[:], 1.0)
            ones_row = cpool.tile([1, 128], F32R, tag="or")
            nc.scalar.copy(ones_row[:], ones_row_f[:])
            ones_col_bf = cpool.tile([128, 1], BF16, tag="ocb")
            nc.scalar.copy(ones_col_bf[:], ones_col_f[:])
            eps_sb = cpool.tile([1, 1], F32, tag="eps")
            nc.vector.memset(eps_sb[:], EPS)
            s0_sb = []
            for mt in range(4):
                t = cpool.tile([128, 1], F32, tag=f"s0{mt}", name=f"s0{mt}")
                nc.vector.tensor_scalar_add(t[:], cw_sb[mt][:, 0:1], 1.0)
                s0_sb.append(t)

            roped = {}
            vaug = {}
            rstdkT = {}
            for b in range(B):
                for mt in range(3):
                    roped[(b, mt)] = pers.tile([128, S], BF16,
                                               tag=f"roped{b}{mt}",
                                               name=f"roped{b}{mt}")
                vaug[b] = pers.tile([128, NI * 128], BF16, tag=f"vaug{b}",
                                    name=f"vaug{b}")
                rstdkT[b] = pers.tile([128, NI], F32, tag=f"rstdkT{b}",
                                      name=f"rstdkT{b}")

            srt_d = {b: dram.tile([NI, 128], BF16, tag=f"srtd{b}",
                                  name=f"srt_d{b}") for b in range(B)}
            a2a_in = [dram.tile([1024, 512], BF16, tag=f"a2ai{h}",
                                name=f"a2a_in{h}") for h in range(2)]
            a2a_out = [dram.tile([1024, 512], BF16, tag=f"a2ao{h}",
                                 name=f"a2a_out{h}") for h in range(2)]

            rope_cm = tc.tile_pool(name="ropes", bufs=1)
            rpool = rope_cm.__enter__()
            prep_cm = tc.tile_pool(name="prep", bufs=1)
            prep = prep_cm.__enter__()
            ropes = {}

            def qkv_batch(b, bw, cn, sq, hv, hv0=None):
                hsrc = (hsT.ap()[:, b * S:(b + 1) * S]
                        .rearrange("(k p) s -> p k s", p=128))
                if hv0 is None:
                    for pc in range(4):
                        nc.scalar.dma_start(
                            hv[32 * pc:32 * (pc + 1), 0, :],
                            hsrc[32 * pc:32 * (pc + 1), 0, :])
                    k0 = 1
                else:
                    k0 = 4
                for k in range(k0, 16):
                    nc.scalar.dma_start(hv[:, k, :], hsrc[:, k, :])

                def hsv(k):
                    return hv0[:, k, :] if (hv0 is not None and k < 4) \
                        else hv[:, k, :]
                if b == 0:
                    for nm, t in (("Ak", ropeAk), ("Bk", ropeBk),
                                  ("Aq", ropeAq), ("Bq", ropeBq)):
                        rt = rpool.tile([DH, S], BF16, tag=f"rope{nm}",
                                        name=f"rope{nm}")
                        nc.scalar.dma_start(rt[:], t.ap())
                        ropes[nm] = rt

                with tc.tile_pool(name=f"qps{b}", bufs=2,
                                  space="PSUM") as qps:
                    for mt in MTORD:
                        psums = [qps.tile([128, 512], F32, tag=f"q{n}",
                                          name=f"q{n}") for n in range(NCH)]
                        for k in range(16):
                            wt_k = prep.tile([128, 128], BF16, tag="wtk",
                                             bufs=8, name="wt_k")
                            nc.sync.dma_start(
                                wt_k[:],
                                wT.ap()[128 * k:128 * (k + 1),
                                        128 * mt:128 * (mt + 1)])
                            hk = hsv(k)
                            for n in range(NCH):
                                nc.tensor.matmul(
                                    psums[n][:], wt_k[:],
                                    hk[:, 512 * n:512 * (n + 1)],
                                    start=(k == 0), stop=(k == 15))
                        raw = bw.tile([128, S], BF16, tag=f"raw{b}",
                                      bufs=1, name="raw")
                        c = cn[mt]
                        for n in range(NCH):
                            sl = slice(512 * n, 512 * (n + 1))
                            nc.scalar.copy(raw[:, sl], psums[n][:])
                            nc.scalar.activation(c[:, sl], psums[n][:],
                                                 AF.Copy,
                                                 scale=s0_sb[mt][:])
                        for k in range(1, K_CONV):
                            nc.vector.scalar_tensor_tensor(
                                c[:, k:S], raw[:, 0:S - k],
                                cw_sb[mt][:, k:k + 1], c[:, k:S],
                                ALU.mult, ALU.add)
                        if mt != 3:
                            nc.vector.tensor_mul(sq[mt][:], c[:], c[:])

            def norm_rope_batch(b, cn, sq):
                va = vaug[b]
                with tc.tile_pool(name=f"nps{b}", bufs=2,
                                  space="PSUM") as nps, \
                     tc.tile_pool(name=f"bps{b}", bufs=2,
                                  space="PSUM") as bps, \
                     tc.tile_pool(name=f"tps{b}", bufs=2,
                                  space="PSUM") as tps:
                    for i in range(NI):
                        tp = tps.tile([128, 128], BF16, tag="tp",
                                      name="tp")
                        nc.tensor.transpose(tp[:],
                                            cn[3][:, 128 * i:128 * (i + 1)],
                                            ident_sb[:])
                        nc.scalar.copy(va[:, 128 * i:128 * (i + 1)], tp[:])
                    srtk = prep.tile([1, S], BF16, tag="srtk", name="srtk")
                    for mt in (2, 0, 1):
                        is_q = mt < 2
                        recq = []
                        for cch in range(NCH):
                            sl = slice(512 * cch, 512 * (cch + 1))
                            sp = nps.tile([1, 512], F32, tag="ssq")
                            nc.tensor.matmul(sp[:], ones_col_bf[:],
                                             sq[mt][:, sl],
                                             start=True, stop=True)
                            if is_q:
                                srt = prep.tile([1, 512], F32R, tag="srtq",
                                                bufs=2, name="srt")
                                nc.scalar.activation(
                                    srt[:], sp[:], AF.Sqrt,
                                    bias=eps_sb[:], scale=1.0 / DH)
                                bp = bps.tile([128, 512], F32, tag="bcp")
                                nc.tensor.matmul(bp[:], ones_row[:],
                                                 srt[:], start=True,
                                                 stop=True)
                                rq = prep.tile([128, 512], F32, tag="recq",
                                               bufs=4, name="rq")
                                nc.vector.reciprocal_approx_fast(rq[:],
                                                                 bp[:])
                                recq.append(rq)
                            else:
                                nc.scalar.activation(
                                    srtk[:, sl], sp[:], AF.Sqrt,
                                    bias=eps_sb[:], scale=1.0 / DH)
                        x = cn[mt]
                        A_ = ropes["Aq"] if is_q else ropes["Ak"]
                        B_ = ropes["Bq"] if is_q else ropes["Bk"]
                        sh = prep.tile([128, S], BF16, tag="sh", bufs=1,
                                       name="sh")
                        nc.gpsimd.dma_start(sh[0:64, :], x[64:128, :])
                        nc.gpsimd.dma_start(sh[64:128, :], x[0:64, :])
                        nc.vector.tensor_mul(sh[:], sh[:], B_[:])
                        tm = prep.tile([128, S], BF16, tag="tm", bufs=1,
                                       name="tm")
                        nc.vector.tensor_mul(tm[:], x[:], A_[:])
                        ro = roped[(b, mt)]
                        if is_q:
                            nc.vector.tensor_add(tm[:], tm[:], sh[:])
                            for cch in range(NCH):
                                sl = slice(512 * cch, 512 * (cch + 1))
                                nc.vector.tensor_mul(ro[:, sl], tm[:, sl],
                                                     recq[cch][:])
                        else:
                            nc.vector.tensor_add(ro[:], tm[:], sh[:])
                            nc.scalar.dma_start(srt_d[b][:], srtk[:])
                            srtkT = prep.tile([128, NI], BF16, tag="srtkT",
                                              name="srtkT")
                            nc.scalar.dma_start(
                                srtkT[:],
                                srt_d[b][:].rearrange("i p -> p i"))
                            nc.vector.reciprocal(rstdkT[b][:], srtkT[:])

            def attn_bh(h, b):
                KT = roped[(b, 2)]
                QT = roped[(b, h)]
                va = vaug[b]
                rkt = rstdkT[b]
                with tc.tile_pool(name=f"pp{h}{b}", bufs=1) as ppool, \
                     tc.tile_pool(name=f"sc{h}{b}", bufs=1,
                                  space="PSUM") as scps, \
                     tc.tile_pool(name=f"pv{h}{b}", bufs=1,
                                  space="PSUM") as pvps, \
                     tc.tile_pool(name=f"st{h}{b}", bufs=2) as stp:
                    pt = ppool.tile([128, PTOT], BF16, tag="pk", name="pk")

                    def pslice(i, j, off):
                        a = poff[i] + 512 * j + off - 128 * i
                        return pt[:, a:a + 512 - off]

                    ots, dsbs = {}, {}

                    def bmms(j):
                        jmax = 4 * j + 3
                        ot = pvps.tile([128, 512], F32, tag="ot", bufs=2,
                                       name="ot")
                        for i in range(jmax + 1):
                            off = max(0, 128 * i - 512 * j)
                            nc.tensor.matmul(
                                ot[:, off:512],
                                va[:, 128 * i:128 * (i + 1)],
                                pslice(i, j, off),
                                start=(i == 0), stop=(i == jmax))
                        den = pvps.tile([1, 512], F32, tag="dn", bufs=1,
                                        name="den")
                        for i in range(jmax + 1):
                            off = max(0, 128 * i - 512 * j)
                            nc.tensor.matmul(
                                den[:, off:512], ones_col_bf[:],
                                pslice(i, j, off),
                                start=(i == 0), stop=(i == jmax))
                        dsb = stp.tile([1, 512], F32R, tag="dsb", bufs=2,
                                       name="dsb")
                        nc.scalar.copy(dsb[:], den[:])
                        ots[j] = ot
                        dsbs[j] = dsb

                    def ep(j):
                        bde = pvps.tile([128, 512], F32, tag="bd", bufs=1,
                                        name="bde")
                        nc.tensor.matmul(bde[:], ones_row[:], dsbs[j][:],
                                         start=True, stop=True)
                        brc = stp.tile([128, 512], F32, tag="brc", bufs=2,
                                       name="brc")
                        nc.vector.reciprocal_approx_fast(brc[:], bde[:])
                        asb = stp.tile([128, 512], BF16, tag="asb",
                                       bufs=2, name="asb")
                        nc.vector.tensor_mul(asb[:], ots[j][:], brc[:])
                        nc.sync.dma_start(
                            a2a_in[h][128 * (4 * b + j):
                                      128 * (4 * b + j + 1), :],
                            asb[:])

                    for g in range(NCH):
                        for i in range(4 * g, 4 * g + 4):
                            lo = 128 * i
                            sc = scps.tile([128, 2048], F32, tag="sc",
                                           name="sc")
                            for n in range(lo // 512, NCH):
                                c0 = max(lo, 512 * n)
                                nc.tensor.matmul(
                                    sc[:, c0:512 * (n + 1)],
                                    KT[:, lo:lo + 128],
                                    QT[:, c0:512 * (n + 1)],
                                    start=True, stop=True)
                            nc.vector.tensor_add(
                                sc[:, lo:lo + 128],
                                sc[:, lo:lo + 128], mask_sb[:])
                            nc.scalar.activation(
                                pt[:, poff[i]:poff[i] + S - lo],
                                sc[:, lo:S], AF.Exp,
                                scale=rkt[:, i:i + 1])
                        if g > 0:
                            ep(g - 1)
                        bmms(g)
                    ep(NCH - 1)

            # ================== phase schedule ==================
            # batch 0
            cn0_cm = tc.tile_pool(name="cn0", bufs=1)
            bw0 = cn0_cm.__enter__()
            cn0 = {mt: bw0.tile([128, S], BF16, tag=f"cn0{mt}",
                                name=f"cn0{mt}") for mt in range(4)}
            sq0_cm = tc.tile_pool(name="sq0", bufs=1)
            sw0 = sq0_cm.__enter__()
            sq0 = {mt: sw0.tile([128, S], BF16, tag=f"sq0{mt}",
                                name=f"sq0{mt}") for mt in range(3)}
            hs0_cm = tc.tile_pool(name="hs0", bufs=1)
            hs0_pool = hs0_cm.__enter__()
            hs0 = hs0_pool.tile([128, 16 * S], BF16, tag="hs0", name="hs0")
            qkv_batch(0, bw0, cn0, sq0,
                      hs0[:].rearrange("p (k s) -> p k s", s=S))
            hs0_cm.__exit__(None, None, None)
            norm_rope_batch(0, cn0, sq0)
            sq0_cm.__exit__(None, None, None)
            cn0_cm.__exit__(None, None, None)

            hsk0_cm = tc.tile_pool(name="hsk0", bufs=1)
            hsk0_pool = hsk0_cm.__enter__()
            hsk0 = hsk0_pool.tile([128, 4 * S], BF16, tag="hsk0",
                                  name="hsk0")
            hk0 = hsk0[:].rearrange("p (k s) -> p k s", s=S)
            hsrc1 = (hsT.ap()[:, S:2 * S]
                     .rearrange("(k p) s -> p k s", p=128))
            for k in range(4):
                nc.scalar.dma_start(hk0[:, k, :], hsrc1[:, k, :])

            attn_bh(0, 0)
            attn_bh(1, 0)

            # batch 1
            cn1_cm = tc.tile_pool(name="cn1", bufs=1)
            bw1 = cn1_cm.__enter__()
            cn1 = {mt: bw1.tile([128, S], BF16, tag=f"cn1{mt}",
                                name=f"cn1{mt}") for mt in range(4)}
            sq1_cm = tc.tile_pool(name="sq1", bufs=1)
            sw1 = sq1_cm.__enter__()
            sq1 = {mt: sw1.tile([128, S], BF16, tag=f"sq1{mt}",
                                name=f"sq1{mt}") for mt in range(3)}
            hs1_cm = tc.tile_pool(name="hs1", bufs=1)
            hs1_pool = hs1_cm.__enter__()
            hs1 = hs1_pool.tile([128, 16 * S], BF16, tag="hs1", name="hs1")
            qkv_batch(1, bw1, cn1, sq1,
                      hs1[:].rearrange("p (k s) -> p k s", s=S), hv0=hk0)
            hs1_cm.__exit__(None, None, None)
            norm_rope_batch(1, cn1, sq1)
            sq1_cm.__exit__(None, None, None)
            cn1_cm.__exit__(None, None, None)
            hsk0_cm.__exit__(None, None, None)
            prep_cm.__exit__(None, None, None)
            rope_cm.__exit__(None, None, None)

            # Wo resident + av staging
            wo_cm = tc.tile_pool(name="wo", bufs=1)
            wo_pool = wo_cm.__enter__()
            wo_sb = wo_pool.tile([128, 16 * D], BF16, tag="wo",
                                 name="wo_sb")
            wov = wo_sb[:].rearrange("p (k c) -> p k c", c=D)
            wsrc = woT.ap().rearrange("(k p) c -> p k c", p=128)
            for g in range(16):
                nc.sync.dma_start(wov[:, g, :], wsrc[:, g, :])
            opool_cm = tc.tile_pool(name="opool", bufs=1)
            opool = opool_cm.__enter__()
            av = []

            attn_bh(0, 1)
            nc.gpsimd.collective_compute(
                "AllToAll", ALU.bypass,
                replica_groups=[list(range(N_CORES))],
                ins=[a2a_in[0].opt()], outs=[a2a_out[0].opt()],
                cc_dim="Partition")
            at0 = opool.tile([128, 8 * 512], BF16, tag="av0", name="av0")
            av0v = at0[:].rearrange("p (k s) -> p k s", s=512)
            a2o0 = a2a_out[0][:].rearrange("(k p) s -> p k s", p=128)
            for sblk in range(8):
                nc.sync.dma_start(av0v[:, sblk, :], a2o0[:, sblk, :])
            av.append(av0v)

            attn_bh(1, 1)
            nc.gpsimd.collective_compute(
                "AllToAll", ALU.bypass,
                replica_groups=[list(range(N_CORES))],
                ins=[a2a_in[1].opt()], outs=[a2a_out[1].opt()],
                cc_dim="Partition")
            at1 = opool.tile([128, 8 * 512], BF16, tag="av1", name="av1")
            av1v = at1[:].rearrange("p (k s) -> p k s", s=512)
            a2o1 = a2a_out[1][:].rearrange("(k p) s -> p k s", p=128)
            for sblk in range(8):
                nc.sync.dma_start(av1v[:, sblk, :], a2o1[:, sblk, :])
            av.append(av1v)

            # ====================== out projection ====================
            with tc.tile_pool(name="ops", bufs=2, space="PSUM") as ops:
                for pair in ((0, 1), (2, 3)):
                    pso = {mp: [ops.tile([128, 512], F32,
                                         tag=f"po{pi * NCH + n}", bufs=1,
                                         name=f"po{pi}{n}")
                                for n in range(NCH)]
                           for pi, mp in enumerate(pair)}
                    for g in [0, 2, 4, 6, 8, 10, 12, 14,
                              1, 3, 5, 7, 9, 11, 13, 15]:
                        for mp in pair:
                            stat = av[g % 2][:, g // 2,
                                             128 * mp:128 * (mp + 1)]
                            for n in range(NCH):
                                nc.tensor.matmul(
                                    pso[mp][n][:], stat,
                                    wov[:, g, 512 * n:512 * (n + 1)],
                                    start=(g == 0), stop=(g == 15))
                    for mp in pair:
                        for n in range(NCH):
                            os_t = opool.tile([128, 512], F32, tag="osb",
                                              bufs=4, name="os_t")
                            nc.scalar.copy(os_t[:], pso[mp][n][:])
                            nc.sync.dma_start(
                                out.ap()[128 * mp:128 * (mp + 1),
                                         512 * n:512 * (n + 1)], os_t[:])
            opool_cm.__exit__(None, None, None)
            wo_cm.__exit__(None, None, None)

    nc.compile()
    return nc


_NC_CACHE = None


def _get_nc():
    global _NC_CACHE
    if _NC_CACHE is None:
        _NC_CACHE = _build()
    return _NC_CACHE


def _host_prep(inputs):
    hs = np.asarray(inputs["hidden_states"], dtype=np.float32)
    Wq = np.asarray(inputs["Wq"], dtype=np.float32)
    Wk = np.asarray(inputs["Wk"], dtype=np.float32)
    Wv = np.asarray(inputs["Wv"], dtype=np.float32)
    Wo = np.asarray(inputs["Wo"], dtype=np.float32)
    cqw = np.asarray(inputs["canon_q_w"], dtype=np.float32)
    ckw = np.asarray(inputs["canon_k_w"], dtype=np.float32)
    cvw = np.asarray(inputs["canon_v_w"], dtype=np.float32)
    qnw = np.asarray(inputs["q_norm_w"], dtype=np.float32)
    knw = np.asarray(inputs["k_norm_w"], dtype=np.float32)

    bf = ml_dtypes.bfloat16
    hsT = np.ascontiguousarray(
        np.concatenate([hs[0].T, hs[1].T], axis=1)).astype(bf)
    WqT, WkT, WvT = Wq.T, Wk.T, Wv.T
    woT = np.ascontiguousarray(Wo.T).astype(bf)

    inv_freq = 1.0 / (10000.0 ** (np.arange(0, DH, 2, dtype=np.float64) / DH))
    freqs = np.arange(S, dtype=np.float64)[:, None] * inv_freq
    emb = np.concatenate([freqs, freqs], axis=-1)
    cosT, sinT = np.cos(emb).T, np.sin(emb).T

    def make_rope(normw, scale):
        A = cosT * normw[:, None] * scale
        wswap = normw[(np.arange(DH) + 64) % DH]
        sign = np.where(np.arange(DH) < 64, -1.0, 1.0)
        Bc = sinT * wswap[:, None] * sign[:, None] * scale
        return (np.ascontiguousarray(A).astype(bf),
                np.ascontiguousarray(Bc).astype(bf))

    Aq, Bq = make_rope(qnw, SCALE)
    Ak, Bk = make_rope(knw, 1.0)

    p = np.arange(128)[:, None]
    f = np.arange(128)[None, :]
    maskd = np.where(p <= f, 0.0, NEG).astype(np.float32)
    identd = np.eye(128, dtype=np.float32).astype(bf)

    in_maps = []
    for r in range(N_CORES):
        wTc = np.ascontiguousarray(np.concatenate(
            [WqT[:, 256 * r:256 * r + 256],
             WkT[:, 128 * r:128 * r + 128],
             WvT[:, 128 * r:128 * r + 128]], axis=1)).astype(bf)
        cwc = np.ascontiguousarray(np.concatenate(
            [cqw[256 * r:256 * r + 256],
             ckw[128 * r:128 * r + 128],
             cvw[128 * r:128 * r + 128]], axis=0)).astype(np.float32)
        in_maps.append({
            "hsT": hsT, "wT": wTc, "woT": woT, "cw": cwc,
            "ropeAq": Aq, "ropeBq": Bq, "ropeAk": Ak, "ropeBk": Bk,
            "maskd": maskd, "identd": identd,
        })
    return in_maps


def kernel(**inputs):
    nc = _get_nc()
    in_maps = _host_prep(inputs)
    res = run_bass_kernel_spmd(nc, in_maps, core_ids=list(range(N_CORES)))
    full = np.empty((B, S, D), np.float32)
    for r in range(N_CORES):
        full[r // 4, 512 * (r % 4):512 * (r % 4 + 1), :] = res.results[r]["out"]
    return full
